# revision 15
# baseline (speedup 1.0000x reference)
"""Trainium2 Bass kernel for a 2-layer GAT model (GATConv -> ELU -> GATConv -> ELU
-> mean readout -> linear).

Strategy (8 NeuronCores, SPMD), v2 -- batched SWDGE gathers:
  - Partition dst nodes (and their incoming edges) across the 8 cores.
  - Each core computes the dense projection table for its node shard with rows
    [h(256) | as(8) | ad(8) | pad] at a 768B pitch; one AllGather per layer
    replicates it.
  - Edges (sorted by dst, grouped into <=128-node blocks of 8 lo-half + 8
    hi-half chunks of 128 edges) are fetched with ONE dma_gather per table
    half per block (the int16 index limit forces the lo/hi split).  dma_gather
    amortizes the ~1us SWDGE fixed cost over 1024 descriptors where the
    previous per-chunk indirect DMAs paid it per 128.
  - Scores: as rides in the gathered row; ad is fetched per-edge by one
    whole-layer dma_gather (dst-indexed, from a dedicated 256B-pitch ad
    table) that overlaps the AllGather.  e = exp(leaky(as+ad)) is written
    into the gathered row's tail slot so a single 272-wide matmul per chunk
    produces both the weighted aggregation and the softmax denominator.
  - The epilogue divides by the denominator, applies ELU, and either collects
    rows for a single batched dma_scatter_add (layer-1 output -> layer-2
    input) or accumulates the column sum for the mean readout (layer 2).
  - A tiny AllReduce combines the per-core column sums; every core finishes
    the linear head redundantly and writes the [1] output.

All graph-dependent tables (gather indices, slot ids, scatter targets) are
built host-side in numpy; all model FLOPs run on the Trainium cores.
"""

import sys

import numpy as np

for _p in ("/opt/trn_rl_repo",):
    if _p not in sys.path:
        sys.path.insert(0, _p)

from concourse import ap_utils, bass, mybir, tile  # noqa: E402
from concourse.bass_utils import run_bass_kernel_spmd  # noqa: E402

F32 = mybir.dt.float32
BF16 = mybir.dt.bfloat16
I16 = mybir.dt.int16
NP_BF16 = mybir.dt.np(BF16)

N_CORES = 8
NEG_SLOPE = 0.2
NEG_BIG = -1e30
K_CH = 16          # chunks (of 128 edges) per block: 8 lo-half + 8 hi-half
K_HALF = K_CH // 2
PITCH = 384        # table row pitch in bf16 elems (768B, 256B-aligned)
RW = 272           # gathered row: 256 h + 8 as + 8 (ad[src] -> e-score slot)
AD_PITCH = 128     # ad table pitch in bf16 elems (256B)

LEGALIZE_WAITS = True


def _legalize_waits(nc, cap=1):
    """Split multi-wait instructions: the TRN2 engine-instruction encodings hold
    only a limited number of sync-wait commands (walrus: "Too many sync wait
    commands"). Move excess waits onto standalone sequencer EventSemaphore
    instructions inserted just before, on the same engine queue."""
    for bb in nc.main_func.blocks:
        out = []
        n_split = 0
        for ins in bb.instructions:
            si = ins.sync_info
            waits = list(si.on_wait) if si and si.on_wait else []
            if len(waits) <= cap:
                out.append(ins)
                continue
            movable = [
                w for w in waits
                if w.sync_type == "semaphore" and w.wait_mode == "sem-ge-imm"
            ]
            keep = [w for w in waits if w not in movable]
            n_move = min(len(movable), len(waits) - cap)
            for wt in movable[:n_move]:
                ev = mybir.InstEventSemaphore(
                    name=f"{ins.name}-w{n_split}", ins=[], outs=[]
                )
                n_split += 1
                ev.engine = ins.engine
                ev.sync_info = mybir.SyncInfo(on_wait=[wt], on_update=[])
                out.append(ev)
            keep.extend(movable[n_move:])
            ins.sync_info = mybir.SyncInfo(
                on_wait=keep, on_update=list(si.on_update) if si.on_update else []
            )
            out.append(ins)
        bb.instructions = out


def _dma_gather(g, out_ap, in_ap, idxs_ap, num_idxs, elem_size, elem_step,
                num_reg=None):
    """nc.gpsimd.dma_gather minus the `elem_size_bytes % 256` assert, which the
    Q7 kernel only needs for transpose mode (non-transpose uses elem_size only
    as the per-index packet byte count; the 256B constraint is on the stride)."""
    assert idxs_ap.dtype == mybir.dt.int16
    assert in_ap.dtype == out_ap.dtype
    assert in_ap.space == bass.MemorySpace.DRAM
    assert idxs_ap.space == bass.MemorySpace.SBUF
    assert out_ap.space == bass.MemorySpace.SBUF
    assert ap_utils.ap_is_contiguous(in_ap.ap[1:])
    assert ap_utils.ap_is_contiguous(out_ap.ap[1:])
    assert ap_utils.ap_is_contiguous(idxs_ap.ap[1:])
    assert in_ap.ap[-1][1] == out_ap.ap[-1][1] == elem_size
    assert out_ap.ap[0][1] * out_ap.ap[1][1] == num_idxs and num_idxs % 128 == 0
    assert in_ap.ap[0][0] == elem_step
    stride_bytes = elem_step * mybir.dt.size(in_ap.dtype)
    stride_bytes_256 = stride_bytes // 256
    assert stride_bytes_256 * 256 == stride_bytes and stride_bytes_256 < 256

    _in_ap = g.lower_ap_dma(in_ap, for_custom_bir_dma=True)
    _idxs_ap = g.lower_ap(idxs_ap)
    _out_ap = g.lower_ap(out_ap)
    nreg = num_reg if num_reg is not None else g.to_reg(num_idxs)
    return g.add_instruction(
        mybir.InstDMAGatherAnt(
            name=g.bass.get_next_instruction_name(),
            ins=[*_in_ap, _idxs_ap, g.lower_val_access(nreg)],
            outs=[_out_ap],
            transpose=False,
            num_idxs=num_idxs,
            elem_size=elem_size,
            stride_bytes_256=stride_bytes_256,
            gen_mode=0,
            single_packet=True,
            queue_num=0,
            sbuf_tokens_per_rank=0,
            sbuf_free_dim_per_rank=0,
            sbuf_free_dim_pad_per_rank=0,
            sbuf_byte_offset=0,
        )
    )


def _wrap16(vals: np.ndarray, n_rows: int = 128) -> np.ndarray:
    """int16 index list -> dma_gather SBUF layout: position i at [i % 16, i // 16],
    replicated across the eight 16-partition groups."""
    assert vals.size % 16 == 0
    w = vals.reshape(-1, 16).T.astype(np.int16)  # [16, n/16]
    return np.tile(w, (n_rows // 16, 1))


# ----------------------------------------------------------------------------
# Host-side graph preprocessing
# ----------------------------------------------------------------------------
def _preprocess(edge_index: np.ndarray, n_nodes: int):
    src = np.asarray(edge_index[0], dtype=np.int64)
    dst = np.asarray(edge_index[1], dtype=np.int64)
    nsh = (n_nodes + N_CORES - 1) // N_CORES
    sr = ((nsh + 1 + 127) // 128) * 128
    sent = sr - 1
    half = (N_CORES // 2) * sr
    assert half <= 32767, "int16 dma_gather index limit"
    cap_half = K_HALF * 128

    owner = np.minimum(dst // nsh, N_CORES - 1)
    src_owner = np.minimum(src // nsh, N_CORES - 1)
    src_grow = src_owner * sr + (src - src_owner * nsh)

    cores = []
    max_blocks = 0
    for k in range(N_CORES):
        lo = k * nsh
        hi_n = min((k + 1) * nsh, n_nodes)
        n_local = hi_n - lo
        m = owner == k
        e_dst = (dst[m] - lo).astype(np.int64)
        e_srcg = src_grow[m]
        order = np.argsort(e_dst, kind="stable")
        e_dst = e_dst[order]
        e_srcg = e_srcg[order]
        is_lo = e_srcg < half
        deg_lo = np.bincount(e_dst[is_lo], minlength=n_local)
        deg_hi = np.bincount(e_dst[~is_lo], minlength=n_local)
        starts = np.zeros(n_local + 1, dtype=np.int64)
        np.cumsum(deg_lo + deg_hi, out=starts[1:])

        blocks = []
        v0 = 0
        cur_lo = cur_hi = cur_n = 0
        for v in range(n_local):
            dl, dh = int(deg_lo[v]), int(deg_hi[v])
            if cur_n + 1 > 128 or cur_lo + dl > cap_half or cur_hi + dh > cap_half:
                blocks.append((v0, cur_n))
                v0 = v
                cur_lo = cur_hi = cur_n = 0
            cur_lo += dl
            cur_hi += dh
            cur_n += 1
        blocks.append((v0, cur_n))
        cores.append(dict(blocks=blocks, e_dst=e_dst, e_srcg=e_srcg,
                          starts=starts, is_lo=is_lo))
        max_blocks = max(max_blocks, len(blocks))

    B = max_blocks
    gidx = np.full((N_CORES, B * K_CH * 128), sent, dtype=np.int16)
    dstl = np.full((N_CORES, B * K_CH * 128), sent, dtype=np.int16)
    scat = np.full((N_CORES, B * 128), sent, dtype=np.int16)
    for k in range(N_CORES):
        info = cores[k]
        starts, e_dst, e_srcg = info["starts"], info["e_dst"], info["e_srcg"]
        for b, (v0, nv) in enumerate(info["blocks"]):
            s, e = starts[v0], starts[v0 + nv]
            bs = e_srcg[s:e]
            bd = e_dst[s:e]
            lo_m = bs < half
            base = b * K_CH * 128
            for hside, mm in ((0, lo_m), (1, ~lo_m)):
                g = bs[mm] - (half if hside else 0)
                d = bd[mm]
                o = np.argsort(g, kind="stable")
                g, d = g[o], d[o]
                ne = g.size
                assert ne <= cap_half
                off = base + hside * cap_half
                gidx[k, off:off + ne] = g.astype(np.int16)
                dstl[k, off:off + ne] = d.astype(np.int16)
                # sentinel-padded tail keeps gidx/dstl at `sent`
                info.setdefault("edges", []).append(
                    (off, ne, (d - v0).astype(np.float64))
                )
            scat[k, b * 128: b * 128 + nv] = v0 + np.arange(nv)

    # per-edge slot table in the gather's [partition, chunk] layout
    drel_pc = np.zeros((N_CORES, 128, B * K_CH), dtype=np.float32)
    for k in range(N_CORES):
        for off, ne, rel in cores[k]["edges"]:
            j = np.arange(ne)
            pos = off + j
            drel_pc[k, pos % 128, pos // 128] = rel

    return dict(
        SR=sr, B=B, NSH=nsh, HALF=half, sent=sent,
        gidx=np.stack([_wrap16(gidx[k]) for k in range(N_CORES)]),
        dstl=np.stack([_wrap16(dstl[k]) for k in range(N_CORES)]),
        scat=np.stack([_wrap16(scat[k]) for k in range(N_CORES)]),
        drel=drel_pc,
    )


# ----------------------------------------------------------------------------
# Bass program
# ----------------------------------------------------------------------------
def _build_program(cfg):
    SR, B = cfg["SR"], cfg["B"]
    F = cfg["F"]            # input features (128)
    D = cfg["D"]            # hidden = heads*chan (256)
    H = cfg["H"]            # heads (8)
    CH = D // H             # channels per head (32)
    HALF = cfg["HALF"]
    G = N_CORES * SR
    n_tiles = SR // 128
    kd = max(1, D // 128)   # K-tiles for layer-2 dense
    C = B * K_CH

    nc = bass.Bass()

    x1T = nc.declare_dram_parameter("x1T", [F, SR], BF16, isOutput=False)
    gidx_p = nc.declare_dram_parameter("gidx", [128, B * 128], I16, isOutput=False)
    dstl_p = nc.declare_dram_parameter("dstl", [128, B * 128], I16, isOutput=False)
    scat_p = nc.declare_dram_parameter("scat", [128, B * 8], I16, isOutput=False)
    drel_p = nc.declare_dram_parameter("drel", [128, C], BF16, isOutput=False)
    w1e_p = nc.declare_dram_parameter("W1e", [F, RW], BF16, isOutput=False)
    w2e_p = nc.declare_dram_parameter("W2e", [D, RW], BF16, isOutput=False)
    iota_p = nc.declare_dram_parameter("iota_rep", [128, K_CH * 128], BF16,
                                       isOutput=False)
    sent_p = nc.declare_dram_parameter("sent_row", [1, RW], BF16, isOutput=False)
    lwg_p = nc.declare_dram_parameter("linw_g", [1, D], F32, isOutput=False)
    lwuw_p = nc.declare_dram_parameter("linw_uw", [1, 2], F32, isOutput=False)
    uw_p = nc.declare_dram_parameter("uw", [1, 2], F32, isOutput=False)
    lb_p = nc.declare_dram_parameter("lin_b", [1, 1], F32, isOutput=False)
    out_p = nc.declare_dram_parameter("out", [1, 1], F32, isOutput=True)

    hext_own = [nc.dram_tensor(f"hext{i}_own", [SR, PITCH], BF16) for i in (1, 2)]
    hext_full = [
        nc.dram_tensor(f"hext{i}_full", [G, PITCH], BF16, addr_space="Shared")
        for i in (1, 2)
    ]
    adp = [nc.dram_tensor(f"adp{i}", [SR, AD_PITCH], BF16) for i in (1, 2)]
    x2_dram = nc.dram_tensor("x2", [SR, D], BF16)
    cs_in = nc.dram_tensor("cs_in", [1, D], F32)
    cs_out = nc.dram_tensor("cs_out", [1, D], F32, addr_space="Shared")

    rg = [list(range(N_CORES))]

    # SWDGE ops are limited to ~1024 indices (descriptor-ring capacity);
    # every dma_gather/dma_scatter_add below stays at <=1024.
    r_blk = nc.gpsimd.alloc_register("r_blk")
    nc.gpsimd.reg_mov(r_blk, K_HALF * 128)
    sc_tail = (B % 8) * 128
    r_tail = None
    if sc_tail:
        r_tail = nc.gpsimd.alloc_register("r_tail")
        nc.gpsimd.reg_mov(r_tail, sc_tail)

    with tile.TileContext(nc) as tc:
        with (
            tc.tile_pool(name="const", bufs=1) as cp,
            tc.tile_pool(name="dstg", bufs=3) as dstgp,
            tc.tile_pool(name="gblk", bufs=3) as gp,
            tc.tile_pool(name="sS", bufs=2) as sp_,
            tc.tile_pool(name="scc", bufs=2) as scp,
            tc.tile_pool(name="sce", bufs=2) as sep,
            tc.tile_pool(name="ep", bufs=2) as epp,
            tc.tile_pool(name="x2s", bufs=2) as x2p,
            tc.tile_pool(name="fin", bufs=1) as fp_,
            tc.tile_pool(name="psA", bufs=2, space="PSUM") as psA,
            tc.tile_pool(name="psO", bufs=2, space="PSUM") as psO,
            tc.tile_pool(name="psC", bufs=1, space="PSUM") as psC,
        ):
            # ---- constants -------------------------------------------------
            x1T_sb = cp.tile([F, SR], BF16, tag="x1T")
            nc.sync.dma_start(out=x1T_sb[:], in_=x1T[:])
            gidx_sb = cp.tile([128, B * 128], I16, tag="gidx")
            nc.sync.dma_start(out=gidx_sb[:], in_=gidx_p[:])
            dstl_sb = cp.tile([128, B * 128], I16, tag="dstl")
            nc.sync.dma_start(out=dstl_sb[:], in_=dstl_p[:])
            scat_sb = cp.tile([128, B * 8], I16, tag="scat")
            nc.sync.dma_start(out=scat_sb[:], in_=scat_p[:])
            drel_sb = cp.tile([128, C], BF16, tag="drel")
            nc.sync.dma_start(out=drel_sb[:], in_=drel_p[:])
            w1e_sb = cp.tile([F, RW], BF16, tag="w1e")
            nc.sync.dma_start(out=w1e_sb[:], in_=w1e_p[:])
            w2e_sb = []
            for q in range(kd):
                wt = cp.tile([128, RW], BF16, tag=f"w2e{q}")
                nc.sync.dma_start(out=wt[:], in_=w2e_p[q * 128:(q + 1) * 128, :])
                w2e_sb.append(wt)
            iota_sb = cp.tile([128, K_CH * 128], BF16, tag="iota")
            nc.sync.dma_start(out=iota_sb[:], in_=iota_p[:])
            sent_sb = cp.tile([1, RW], BF16, tag="sent")
            nc.sync.dma_start(out=sent_sb[:], in_=sent_p[:])
            ones_sb = cp.tile([128, 1], BF16, tag="ones")
            nc.vector.memset(ones_sb[:], 1.0)
            adsent_sb = cp.tile([1, H], BF16, tag="adsent")
            nc.vector.memset(adsent_sb[:], 0.0)
            lwg_sb = cp.tile([1, D], F32, tag="lwg")
            nc.sync.dma_start(out=lwg_sb[:], in_=lwg_p[:])
            lwuw_sb = cp.tile([1, 2], F32, tag="lwuw")
            nc.sync.dma_start(out=lwuw_sb[:], in_=lwuw_p[:])
            uw_sb = cp.tile([1, 2], F32, tag="uw")
            nc.sync.dma_start(out=uw_sb[:], in_=uw_p[:])
            lb_sb = cp.tile([1, 1], F32, tag="lb")
            nc.sync.dma_start(out=lb_sb[:], in_=lb_p[:])

            adE_sb = []
            for i in (1, 2):
                adE_t = cp.tile([128, B * 128], BF16, tag=f"adE{i}", name=f"adE{i}")
                adE_sb.append(adE_t)
            xs_all = cp.tile([128, B * D], BF16, tag="xs_all")

            # zero x2 (scatter-add target; pad rows must stay zero)
            zt = cp.tile([128, D], BF16, tag="zpad")
            nc.vector.memset(zt[:], 0.0)
            for t in range(n_tiles):
                nc.sync.dma_start(
                    out=x2_dram[t * 128:(t + 1) * 128, :], in_=zt[:]
                )

            csum_ps = psC.tile([1, D], F32, tag="cs")
            x2T_sb = None

            for layer in range(2):
                # ---- dense ------------------------------------------------
                for t in range(n_tiles):
                    ps = psA.tile([128, RW], F32, tag="ps")
                    if layer == 0:
                        nc.tensor.matmul(
                            out=ps[:],
                            lhsT=x1T_sb[:, t * 128:(t + 1) * 128],
                            rhs=w1e_sb[:],
                            start=True, stop=True,
                        )
                    else:
                        for q in range(kd):
                            nc.tensor.matmul(
                                out=ps[:],
                                lhsT=x2T_sb[q][:, t * 128:(t + 1) * 128],
                                rhs=w2e_sb[q][:],
                                start=(q == 0), stop=(q == kd - 1),
                            )
                    stg = dstgp.tile([128, RW], BF16, tag="stg")
                    nc.vector.tensor_copy(out=stg[:], in_=ps[:])
                    r1 = SR - 1 if t == n_tiles - 1 else (t + 1) * 128
                    nr = r1 - t * 128
                    nc.sync.dma_start(
                        out=hext_own[layer][t * 128:r1, 0:RW], in_=stg[0:nr, :]
                    )
                    nc.sync.dma_start(
                        out=adp[layer][t * 128:r1, 0:H],
                        in_=stg[0:nr, RW - H:RW],
                    )
                    if t == n_tiles - 1:
                        nc.sync.dma_start(
                            out=hext_own[layer][SR - 1: SR, 0:RW], in_=sent_sb[:]
                        )
                        nc.sync.dma_start(
                            out=adp[layer][SR - 1: SR, 0:H], in_=adsent_sb[:]
                        )

                # ---- AllGather --------------------------------------------
                nc.gpsimd.collective_compute(
                    "AllGather",
                    mybir.AluOpType.bypass,
                    ins=[hext_own[layer][:]],
                    outs=[hext_full[layer][:]],
                    replica_groups=rg,
                )

                # ---- edge pass --------------------------------------------
                for b in range(B):
                    # per-edge ad gather for this block (overlaps AllGather
                    # for early blocks: only depends on the local adp table)
                    for j in range(2):
                        _dma_gather(
                            nc.gpsimd,
                            out_ap=adE_sb[layer][
                                :, b * 128 + j * 64: b * 128 + (j + 1) * 64
                            ].rearrange("p (c h) -> p c h", h=H),
                            in_ap=adp[layer][:, 0:H],
                            idxs_ap=dstl_sb[
                                :, b * 128 + j * 64: b * 128 + (j + 1) * 64
                            ],
                            num_idxs=K_HALF * 128,
                            elem_size=H,
                            elem_step=AD_PITCH,
                            num_reg=r_blk,
                        )
                    gblk = gp.tile([128, K_CH * RW], BF16, tag="gblk")
                    for hs in range(2):
                        _dma_gather(
                            nc.gpsimd,
                            out_ap=gblk[
                                :, hs * K_HALF * RW:(hs + 1) * K_HALF * RW
                            ].rearrange("p (c w) -> p c w", w=RW),
                            in_ap=hext_full[layer][
                                hs * HALF:(hs + 1) * HALF, 0:RW
                            ],
                            idxs_ap=gidx_sb[
                                :, b * 128 + hs * 64: b * 128 + (hs + 1) * 64
                            ],
                            num_idxs=K_HALF * 128,
                            elem_size=RW,
                            elem_step=PITCH,
                            num_reg=r_blk,
                        )
                    # one-hot S for the whole block
                    s_all = sp_.tile([128, K_CH * 128], BF16, tag="s_all")
                    nc.vector.tensor_tensor(
                        out=s_all[:].rearrange("p (g e) -> p g e", e=128),
                        in0=iota_sb[:].rearrange("p (g e) -> p g e", e=128),
                        in1=drel_sb[:, b * K_CH:(b + 1) * K_CH]
                        .unsqueeze(-1).to_broadcast([128, K_CH, 128]),
                        op=mybir.AluOpType.is_equal,
                    )
                    # scores: e = exp(leaky(as + ad))
                    gv = gblk[:].rearrange("p (g w) -> p g w", w=RW)
                    scc = scp.tile([128, K_CH * H], BF16, tag="scc")
                    nc.vector.tensor_tensor(
                        out=scc[:].rearrange("p (g h) -> p g h", h=H),
                        in0=gv[:, :, D:D + H],
                        in1=adE_sb[layer][:, b * 128:(b + 1) * 128]
                        .rearrange("p (g h) -> p g h", h=H),
                        op=mybir.AluOpType.add,
                    )
                    t2 = scp.tile([128, K_CH * H], BF16, tag="t2")
                    nc.vector.tensor_scalar_mul(
                        out=t2[:], in0=scc[:], scalar1=NEG_SLOPE
                    )
                    nc.vector.tensor_tensor(
                        out=scc[:], in0=scc[:], in1=t2[:], op=mybir.AluOpType.max
                    )
                    sce = sep.tile([128, K_CH * D], BF16, tag="sce")
                    nc.scalar.activation(
                        out=sce[:].rearrange("p (g h c) -> p g h c", h=H, c=CH),
                        in_=scc[:].rearrange("p (g h) -> p g h", h=H)
                        .unsqueeze(-1).to_broadcast([128, K_CH, H, CH]),
                        func=mybir.ActivationFunctionType.Exp,
                    )
                    nc.scalar.activation(
                        out=gv[:, :, D + H:RW],
                        in_=scc[:].rearrange("p (g h) -> p g h", h=H),
                        func=mybir.ActivationFunctionType.Exp,
                    )
                    nc.vector.tensor_tensor(
                        out=gv[:, :, 0:D],
                        in0=gv[:, :, 0:D],
                        in1=sce[:].rearrange("p (g c) -> p g c", c=D),
                        op=mybir.AluOpType.mult,
                    )
                    # aggregation + denominator in one matmul per chunk
                    ps_o = psO.tile([128, RW], F32, tag="ps_o")
                    for kk in range(K_CH):
                        nc.tensor.matmul(
                            out=ps_o[:],
                            lhsT=s_all[:, kk * 128:(kk + 1) * 128],
                            rhs=gblk[:, kk * RW:(kk + 1) * RW],
                            start=(kk == 0), stop=(kk == K_CH - 1),
                        )
                    # ---- epilogue ----------------------------------------
                    rd = epp.tile([128, H], F32, tag="rd")
                    nc.vector.tensor_scalar_add(
                        out=rd[:], in0=ps_o[:, D + H:RW], scalar1=1e-16
                    )
                    nc.vector.reciprocal(out=rd[:], in_=rd[:])
                    rde = epp.tile([128, D], F32, tag="rde")
                    nc.scalar.activation(
                        out=rde[:],
                        in_=rd[:].unsqueeze(-1).to_broadcast([128, H, CH]),
                        func=mybir.ActivationFunctionType.Copy,
                    )
                    st = epp.tile([128, D], F32, tag="st")
                    nc.vector.tensor_tensor(
                        out=st[:], in0=ps_o[:, 0:D], in1=rde[:],
                        op=mybir.AluOpType.mult,
                    )
                    tm = epp.tile([128, D], F32, tag="tm")
                    nc.vector.tensor_scalar_min(out=tm[:], in0=st[:], scalar1=0.0)
                    nc.scalar.activation(
                        out=tm[:], in_=tm[:], func=mybir.ActivationFunctionType.Exp
                    )
                    nc.vector.tensor_scalar(
                        out=st[:], in0=st[:],
                        scalar1=0.0, scalar2=-1.0,
                        op0=mybir.AluOpType.max, op1=mybir.AluOpType.add,
                    )
                    if layer == 0:
                        nc.vector.tensor_tensor(
                            out=xs_all[:, b * D:(b + 1) * D],
                            in0=st[:], in1=tm[:], op=mybir.AluOpType.add,
                        )
                    else:
                        xs = x2p.tile([128, D], BF16, tag="xs")
                        nc.vector.tensor_tensor(
                            out=xs[:], in0=st[:], in1=tm[:], op=mybir.AluOpType.add
                        )
                        nc.tensor.matmul(
                            out=csum_ps[:],
                            lhsT=ones_sb[:],
                            rhs=xs[:],
                            start=(b == 0), stop=(b == B - 1),
                        )

                # ---- layer-1 output scatter + transpose -------------------
                if layer == 0:
                    for j in range((B + 7) // 8):
                        b0, b1 = j * 8, min((j + 1) * 8, B)
                        nb = b1 - b0
                        nc.gpsimd.dma_scatter_add(
                            out_ap=x2_dram[:],
                            in_ap=xs_all[
                                :, b0 * D:b1 * D
                            ].rearrange("p (b d) -> p b d", d=D),
                            idxs_ap=scat_sb[:, b0 * 8:b1 * 8],
                            num_idxs=nb * 128,
                            num_idxs_reg=r_blk if nb == 8 else r_tail,
                            elem_size=D,
                        )
                    x2T_sb = []
                    for q in range(kd):
                        xt = cp.tile([128, SR], BF16, tag=f"x2T{q}")
                        nc.sync.dma_start_transpose(
                            out=xt[:], in_=x2_dram[:, q * 128:(q + 1) * 128]
                        )
                        x2T_sb.append(xt)

            # ---- readout ---------------------------------------------------
            cs_sb = fp_.tile([1, D], F32, tag="cs_sb")
            nc.vector.tensor_copy(out=cs_sb[:], in_=csum_ps[:])
            nc.sync.dma_start(out=cs_in[:], in_=cs_sb[:])
            nc.gpsimd.collective_compute(
                "AllReduce",
                mybir.AluOpType.add,
                ins=[cs_in[:]],
                outs=[cs_out[:]],
                replica_groups=rg,
            )
            cs2 = fp_.tile([1, D], F32, tag="cs2")
            nc.sync.dma_start(out=cs2[:], in_=cs_out[:])
            tg = fp_.tile([1, D], F32, tag="tg")
            acc1 = fp_.tile([1, 1], F32, tag="acc1")
            nc.vector.tensor_tensor(
                out=tg[:], in0=cs2[:], in1=lwg_sb[:], op=mybir.AluOpType.mult
            )
            nc.vector.tensor_reduce(
                out=acc1[:], in_=tg[:], axis=mybir.AxisListType.X,
                op=mybir.AluOpType.add,
            )
            t2f = fp_.tile([1, 2], F32, tag="t2f")
            acc2 = fp_.tile([1, 1], F32, tag="acc2")
            nc.vector.tensor_tensor(
                out=t2f[:], in0=uw_sb[:], in1=lwuw_sb[:], op=mybir.AluOpType.mult
            )
            nc.vector.tensor_reduce(
                out=acc2[:], in_=t2f[:], axis=mybir.AxisListType.X,
                op=mybir.AluOpType.add,
            )
            nc.vector.tensor_tensor(
                out=acc1[:], in0=acc1[:], in1=acc2[:], op=mybir.AluOpType.add
            )
            nc.vector.tensor_tensor(
                out=acc1[:], in0=acc1[:], in1=lb_sb[:], op=mybir.AluOpType.add
            )
            nc.sync.dma_start(out=out_p[:], in_=acc1[:])

    # Extended Q7 instructions (dma_gather/dma_scatter_add) live in loadable
    # libraries; insert the ModifyPoolConfig reloads and encode them to ISA
    # bytes so walrus's visitInstISA can emit them.
    import bass_rust as _bass_rust

    from concourse.library_config import all_libraries, standard
    inst_type_to_lib_mask = {}
    for lib in all_libraries:
        for inst_type in lib.instructions:
            inst_type_to_lib_mask[inst_type] = inst_type_to_lib_mask.get(
                inst_type, 0
            ) | (1 << lib.index)
    _bass_rust.insert_library_loads(
        nc, inst_type_to_lib_mask, len(all_libraries), standard.index
    )
    mybir.codegen_inst_isa_subclasses(nc)

    if LEGALIZE_WAITS:
        _legalize_waits(nc)
    return nc


# ----------------------------------------------------------------------------
# Host-side input assembly
# ----------------------------------------------------------------------------
def _att_matrix(att: np.ndarray) -> np.ndarray:
    Hh, Cc = att.shape
    A = np.zeros((Hh * Cc, Hh), dtype=np.float64)
    for h in range(Hh):
        A[h * Cc:(h + 1) * Cc, h] = att[h]
    return A


def _pack_we(W, a_s, a_d):
    """[W | W@As | W@Ad] -> packed [h(256) | as(8) | ad(8)]."""
    return np.concatenate([W, W @ _att_matrix(a_s), W @ _att_matrix(a_d)], axis=1)


def _make_inputs(prep, cfg, x, u, w, W1, as1, ad1, W2, as2, ad2, lin_w, lin_b):
    SR, NSH = prep["SR"], prep["NSH"]
    F, D, H = cfg["F"], cfg["D"], cfg["H"]
    n_nodes = x.shape[0]

    W1e = _pack_we(W1, as1, ad1).astype(NP_BF16)
    W2e = _pack_we(W2, as2, ad2).astype(NP_BF16)
    iota_rep = np.tile(np.arange(128, dtype=np.float32), (128, K_CH)).astype(
        NP_BF16
    )
    sent_row = np.zeros((1, RW), dtype=np.float32)
    sent_row[0, D:D + H] = NEG_BIG
    linw_g = (lin_w[0, :D] / float(n_nodes)).astype(np.float32).reshape(1, D)
    linw_uw = lin_w[0, D:D + 2].astype(np.float32).reshape(1, 2)
    uwv = np.array([[float(u), float(w)]], dtype=np.float32)
    lbv = np.asarray(lin_b, dtype=np.float32).reshape(1, 1)

    in_maps = []
    for k in range(N_CORES):
        lo = k * NSH
        hi = min(lo + NSH, n_nodes)
        xs = np.zeros((SR, F), dtype=np.float32)
        xs[: hi - lo] = x[lo:hi]
        m = {
            "x1T": np.ascontiguousarray(xs.T).astype(NP_BF16),
            "gidx": prep["gidx"][k],
            "dstl": prep["dstl"][k],
            "scat": prep["scat"][k],
            "drel": prep["drel"][k].astype(NP_BF16),
            "W1e": W1e,
            "W2e": W2e,
            "iota_rep": iota_rep,
            "sent_row": sent_row.astype(NP_BF16),
            "linw_g": linw_g,
            "linw_uw": linw_uw,
            "uw": uwv,
            "lin_b": lbv,
        }
        in_maps.append(m)
    return in_maps


def build_all(x, edge_index, u, w, W1, att_src1, att_dst1, bias1,
              W2, att_src2, att_dst2, bias2, lin_w, lin_b, **_kw):
    n_nodes, F = x.shape
    H, Cc = att_src1.shape
    D = H * Cc
    assert not (np.any(np.asarray(bias1)) or np.any(np.asarray(bias2))), (
        "bias path not implemented (reference uses zero biases)"
    )
    prep = _preprocess(np.asarray(edge_index), n_nodes)
    cfg = dict(
        SR=prep["SR"], B=prep["B"], NSH=prep["NSH"], HALF=prep["HALF"],
        F=F, D=D, H=H,
    )
    nc = _build_program(cfg)
    in_maps = _make_inputs(
        prep, cfg, np.asarray(x, np.float32), u, w,
        np.asarray(W1, np.float64), np.asarray(att_src1, np.float64),
        np.asarray(att_dst1, np.float64),
        np.asarray(W2, np.float64), np.asarray(att_src2, np.float64),
        np.asarray(att_dst2, np.float64),
        np.asarray(lin_w, np.float64), np.asarray(lin_b, np.float64),
    )
    return nc, in_maps


def kernel(**inputs) -> np.ndarray:
    nc, in_maps = build_all(
        inputs["x"], inputs["edge_index"], inputs["u"], inputs["w"],
        inputs["W1"], inputs["att_src1"], inputs["att_dst1"], inputs["bias1"],
        inputs["W2"], inputs["att_src2"], inputs["att_dst2"], inputs["bias2"],
        inputs["lin_w"], inputs["lin_b"],
    )
    res = run_bass_kernel_spmd(nc, in_maps, core_ids=list(range(N_CORES)))
    return res.results[0]["out"].reshape(1).astype(np.float32)


# revision 27
# speedup vs baseline: 2.1289x; 2.1289x over previous
"""Trainium2 Bass kernel for a 2-layer GAT model (GATConv -> ELU -> GATConv -> ELU
-> mean readout -> linear).

Strategy (8 NeuronCores, SPMD), v2 -- batched SWDGE gathers:
  - Partition dst nodes (and their incoming edges) across the 8 cores.
  - Each core computes the dense projection table for its node shard with rows
    [h(256) | as(8) | ad(8) | pad] at a 768B pitch; one AllGather per layer
    replicates it.
  - Edges (sorted by dst, grouped into <=128-node blocks of 8 lo-half + 8
    hi-half chunks of 128 edges) are fetched with ONE dma_gather per table
    half per block (the int16 index limit forces the lo/hi split).  dma_gather
    amortizes the ~1us SWDGE fixed cost over 1024 descriptors where the
    previous per-chunk indirect DMAs paid it per 128.
  - Scores: as rides in the gathered row; ad is fetched per-edge by one
    whole-layer dma_gather (dst-indexed, from a dedicated 256B-pitch ad
    table) that overlaps the AllGather.  e = exp(leaky(as+ad)) is written
    into the gathered row's tail slot so a single 272-wide matmul per chunk
    produces both the weighted aggregation and the softmax denominator.
  - The epilogue divides by the denominator, applies ELU, and either collects
    rows for a single batched dma_scatter_add (layer-1 output -> layer-2
    input) or accumulates the column sum for the mean readout (layer 2).
  - A tiny AllReduce combines the per-core column sums; every core finishes
    the linear head redundantly and writes the [1] output.

All graph-dependent tables (gather indices, slot ids, scatter targets) are
built host-side in numpy; all model FLOPs run on the Trainium cores.
"""

import sys

import numpy as np

for _p in ("/opt/trn_rl_repo",):
    if _p not in sys.path:
        sys.path.insert(0, _p)

from concourse import ap_utils, bass, mybir, tile  # noqa: E402
from concourse.bass_utils import run_bass_kernel_spmd  # noqa: E402

F32 = mybir.dt.float32
BF16 = mybir.dt.bfloat16
I16 = mybir.dt.int16
U8 = mybir.dt.uint8
NP_BF16 = mybir.dt.np(BF16)

N_CORES = 8
NEG_SLOPE = 0.2
NEG_BIG = -1e30
K_CH = 16          # chunks (of 128 edges) per block: 8 lo-half + 8 hi-half
K_HALF = K_CH // 2
PITCH = 384        # table row pitch in bf16 elems (768B, 256B-aligned)
RW = 272           # gathered row: 256 h + 8 as + 8 (ad[src] -> e-score slot)
AD_PITCH = 128     # ad table pitch in bf16 elems (256B)

LEGALIZE_WAITS = True


def _legalize_waits(nc, cap=1):
    """Split multi-wait instructions: the TRN2 engine-instruction encodings hold
    only a limited number of sync-wait commands (walrus: "Too many sync wait
    commands"). Move excess waits onto standalone sequencer EventSemaphore
    instructions inserted just before, on the same engine queue."""
    for bb in nc.main_func.blocks:
        out = []
        n_split = 0
        for ins in bb.instructions:
            si = ins.sync_info
            waits = list(si.on_wait) if si and si.on_wait else []
            if len(waits) <= cap:
                out.append(ins)
                continue
            movable = [
                w for w in waits
                if w.sync_type == "semaphore" and w.wait_mode == "sem-ge-imm"
            ]
            keep = [w for w in waits if w not in movable]
            n_move = min(len(movable), len(waits) - cap)
            for wt in movable[:n_move]:
                ev = mybir.InstEventSemaphore(
                    name=f"{ins.name}-w{n_split}", ins=[], outs=[]
                )
                n_split += 1
                ev.engine = ins.engine
                ev.sync_info = mybir.SyncInfo(on_wait=[wt], on_update=[])
                out.append(ev)
            keep.extend(movable[n_move:])
            ins.sync_info = mybir.SyncInfo(
                on_wait=keep, on_update=list(si.on_update) if si.on_update else []
            )
            out.append(ins)
        bb.instructions = out


def _dma_gather(g, out_ap, in_ap, idxs_ap, num_idxs, elem_size, elem_step,
                num_reg=None, queue_num=0):
    """nc.gpsimd.dma_gather minus the `elem_size_bytes % 256` assert, which the
    Q7 kernel only needs for transpose mode (non-transpose uses elem_size only
    as the per-index packet byte count; the 256B constraint is on the stride)."""
    assert idxs_ap.dtype == mybir.dt.int16
    assert in_ap.dtype == out_ap.dtype
    assert in_ap.space == bass.MemorySpace.DRAM
    assert idxs_ap.space == bass.MemorySpace.SBUF
    assert out_ap.space == bass.MemorySpace.SBUF
    assert ap_utils.ap_is_contiguous(in_ap.ap[1:])
    assert ap_utils.ap_is_contiguous(out_ap.ap[1:])
    assert ap_utils.ap_is_contiguous(idxs_ap.ap[1:])
    assert in_ap.ap[-1][1] == out_ap.ap[-1][1] == elem_size
    assert out_ap.ap[0][1] * out_ap.ap[1][1] == num_idxs and num_idxs % 128 == 0
    assert in_ap.ap[0][0] == elem_step
    stride_bytes = elem_step * mybir.dt.size(in_ap.dtype)
    stride_bytes_256 = stride_bytes // 256
    assert stride_bytes_256 * 256 == stride_bytes and stride_bytes_256 < 256

    _in_ap = g.lower_ap_dma(in_ap, for_custom_bir_dma=True)
    _idxs_ap = g.lower_ap(idxs_ap)
    _out_ap = g.lower_ap(out_ap)
    nreg = num_reg if num_reg is not None else g.to_reg(num_idxs)
    return g.add_instruction(
        mybir.InstDMAGatherAnt(
            name=g.bass.get_next_instruction_name(),
            ins=[*_in_ap, _idxs_ap, g.lower_val_access(nreg)],
            outs=[_out_ap],
            transpose=False,
            num_idxs=num_idxs,
            elem_size=elem_size,
            stride_bytes_256=stride_bytes_256,
            gen_mode=0,
            single_packet=True,
            queue_num=queue_num,
            sbuf_tokens_per_rank=0,
            sbuf_free_dim_per_rank=0,
            sbuf_free_dim_pad_per_rank=0,
            sbuf_byte_offset=0,
        )
    )


def _wrap16(vals: np.ndarray, n_rows: int = 128) -> np.ndarray:
    """int16 index list -> dma_gather SBUF layout: position i at [i % 16, i // 16],
    replicated across the eight 16-partition groups."""
    assert vals.size % 16 == 0
    w = vals.reshape(-1, 16).T.astype(np.int16)  # [16, n/16]
    return np.tile(w, (n_rows // 16, 1))


# ----------------------------------------------------------------------------
# Host-side graph preprocessing
# ----------------------------------------------------------------------------
def _preprocess(edge_index: np.ndarray, n_nodes: int):
    src = np.asarray(edge_index[0], dtype=np.int64)
    dst = np.asarray(edge_index[1], dtype=np.int64)
    nsh = (n_nodes + N_CORES - 1) // N_CORES
    sr = ((nsh + 1 + 127) // 128) * 128
    sent = sr - 1
    half = (N_CORES // 2) * sr
    assert half <= 32767, "int16 dma_gather index limit"
    cap_half = K_HALF * 128

    owner = np.minimum(dst // nsh, N_CORES - 1)
    src_owner = np.minimum(src // nsh, N_CORES - 1)
    src_grow = src_owner * sr + (src - src_owner * nsh)

    cores = []
    max_blocks = 0
    for k in range(N_CORES):
        lo = k * nsh
        hi_n = min((k + 1) * nsh, n_nodes)
        n_local = hi_n - lo
        m = owner == k
        e_dst = (dst[m] - lo).astype(np.int64)
        e_srcg = src_grow[m]
        order = np.argsort(e_dst, kind="stable")
        e_dst = e_dst[order]
        e_srcg = e_srcg[order]
        is_lo = e_srcg < half
        deg_lo = np.bincount(e_dst[is_lo], minlength=n_local)
        deg_hi = np.bincount(e_dst[~is_lo], minlength=n_local)
        starts = np.zeros(n_local + 1, dtype=np.int64)
        np.cumsum(deg_lo + deg_hi, out=starts[1:])

        blocks = []
        v0 = 0
        cur_lo = cur_hi = cur_n = 0
        for v in range(n_local):
            dl, dh = int(deg_lo[v]), int(deg_hi[v])
            if cur_n + 1 > 128 or cur_lo + dl > cap_half or cur_hi + dh > cap_half:
                blocks.append((v0, cur_n))
                v0 = v
                cur_lo = cur_hi = cur_n = 0
            cur_lo += dl
            cur_hi += dh
            cur_n += 1
        blocks.append((v0, cur_n))
        cores.append(dict(blocks=blocks, e_dst=e_dst, e_srcg=e_srcg,
                          starts=starts, is_lo=is_lo))
        max_blocks = max(max_blocks, len(blocks))

    B = max_blocks
    gidx = np.full((N_CORES, B * K_CH * 128), sent, dtype=np.int16)
    scat = np.full((N_CORES, B * 128), sent, dtype=np.int16)
    for k in range(N_CORES):
        info = cores[k]
        starts, e_dst, e_srcg = info["starts"], info["e_dst"], info["e_srcg"]
        for b, (v0, nv) in enumerate(info["blocks"]):
            s, e = starts[v0], starts[v0 + nv]
            bs = e_srcg[s:e]
            bd = e_dst[s:e]
            lo_m = bs < half
            base = b * K_CH * 128
            for hside, mm in ((0, lo_m), (1, ~lo_m)):
                g = bs[mm] - (half if hside else 0)
                d = bd[mm]
                o = np.argsort(g, kind="stable")
                g, d = g[o], d[o]
                ne = g.size
                assert ne <= cap_half
                off = base + hside * cap_half
                gidx[k, off:off + ne] = g.astype(np.int16)
                # sentinel-padded tail keeps gidx at `sent` (drel at 0)
                info.setdefault("edges", []).append(
                    (off, ne, (d - v0).astype(np.float64))
                )
            scat[k, b * 128: b * 128 + nv] = v0 + np.arange(nv)

    # per-edge slot table, twice: [partition, chunk] for the S build and
    # partition-replicated [*, chunk*128+pos] (uint8) for the S^T build
    drel_pc = np.zeros((N_CORES, 128, B * K_CH), dtype=np.float32)
    drelT = np.zeros((N_CORES, B * K_CH * 128), dtype=np.uint8)
    for k in range(N_CORES):
        for off, ne, rel in cores[k]["edges"]:
            j = np.arange(ne)
            pos = off + j
            drel_pc[k, pos % 128, pos // 128] = rel
            drelT[k, pos] = rel
    drelT_rep = np.stack(
        [np.tile(drelT[k][None, :], (128, 1)) for k in range(N_CORES)]
    )

    return dict(
        SR=sr, B=B, NSH=nsh, HALF=half, sent=sent,
        gidx=np.stack([_wrap16(gidx[k]) for k in range(N_CORES)]),
        scat=np.stack([_wrap16(scat[k]) for k in range(N_CORES)]),
        drel=drel_pc,
        drelT=drelT_rep,
    )


# ----------------------------------------------------------------------------
# Bass program
# ----------------------------------------------------------------------------
def _build_program(cfg):
    SR, B = cfg["SR"], cfg["B"]
    F = cfg["F"]            # input features (128)
    D = cfg["D"]            # hidden = heads*chan (256)
    H = cfg["H"]            # heads (8)
    CH = D // H             # channels per head (32)
    HALF = cfg["HALF"]
    G = N_CORES * SR
    n_tiles = SR // 128
    kd = max(1, D // 128)   # K-tiles for layer-2 dense
    C = B * K_CH

    nc = bass.Bass(num_swdge_queues=4)

    x1T = nc.declare_dram_parameter("x1T", [F, SR], BF16, isOutput=False)
    gidx_p = nc.declare_dram_parameter("gidx", [128, B * 128], I16, isOutput=False)
    scat_p = nc.declare_dram_parameter("scat", [128, B * 8], I16, isOutput=False)
    drel_p = nc.declare_dram_parameter("drel", [128, C], BF16, isOutput=False)
    drelT_p = nc.declare_dram_parameter("drelT", [128, C * 128], U8,
                                        isOutput=False)
    iotac_p = nc.declare_dram_parameter("iota_col", [128, 1], F32,
                                        isOutput=False)
    w1e_p = nc.declare_dram_parameter("W1e", [F, RW], BF16, isOutput=False)
    w2e_p = nc.declare_dram_parameter("W2e", [D, RW], BF16, isOutput=False)
    iota_p = nc.declare_dram_parameter("iota_rep", [128, K_CH * 128], BF16,
                                       isOutput=False)
    sent_p = nc.declare_dram_parameter("sent_row", [1, RW], BF16, isOutput=False)
    lwg_p = nc.declare_dram_parameter("linw_g", [1, D], F32, isOutput=False)
    lwuw_p = nc.declare_dram_parameter("linw_uw", [1, 2], F32, isOutput=False)
    uw_p = nc.declare_dram_parameter("uw", [1, 2], F32, isOutput=False)
    lb_p = nc.declare_dram_parameter("lin_b", [1, 1], F32, isOutput=False)
    out_p = nc.declare_dram_parameter("out", [1, 1], F32, isOutput=True)

    hext_own = [nc.dram_tensor(f"hext{i}_own", [SR, PITCH], BF16) for i in (1, 2)]
    hext_full = [
        nc.dram_tensor(f"hext{i}_full", [G, PITCH], BF16, addr_space="Shared")
        for i in (1, 2)
    ]
    adp = [nc.dram_tensor(f"adp{i}", [SR, AD_PITCH], BF16) for i in (1, 2)]
    x2_dram = nc.dram_tensor("x2", [SR, D], BF16)
    cs_in = nc.dram_tensor("cs_in", [1, D], F32)
    cs_out = nc.dram_tensor("cs_out", [1, D], F32, addr_space="Shared")

    rg = [list(range(N_CORES))]

    # SWDGE ops are limited to ~1024 indices (descriptor-ring capacity);
    # every dma_gather/dma_scatter_add below stays at <=1024.
    r_blk = nc.gpsimd.alloc_register("r_blk")
    nc.gpsimd.reg_mov(r_blk, K_HALF * 128)
    sc_tail = (B % 8) * 128
    r_tail = None
    if sc_tail:
        r_tail = nc.gpsimd.alloc_register("r_tail")
        nc.gpsimd.reg_mov(r_tail, sc_tail)

    with tile.TileContext(nc) as tc:
        with (
            tc.tile_pool(name="const", bufs=1) as cp,
            tc.tile_pool(name="dstg", bufs=3) as dstgp,
            tc.tile_pool(name="gblk", bufs=4) as gp,
            tc.tile_pool(name="sS", bufs=2) as sp_,
            tc.tile_pool(name="dT", bufs=2) as dTp,
            tc.tile_pool(name="psAD", bufs=2, space="PSUM") as psAD,
            tc.tile_pool(name="scc", bufs=2) as scp,
            tc.tile_pool(name="sce", bufs=2) as sep,
            tc.tile_pool(name="ep", bufs=2) as epp,
            tc.tile_pool(name="x2s", bufs=2) as x2p,
            tc.tile_pool(name="fin", bufs=1) as fp_,
            tc.tile_pool(name="psA", bufs=2, space="PSUM") as psA,
            tc.tile_pool(name="psO", bufs=2, space="PSUM") as psO,
            tc.tile_pool(name="psC", bufs=1, space="PSUM") as psC,
        ):
            # ---- constants -------------------------------------------------
            x1T_sb = cp.tile([F, SR], BF16, tag="x1T")
            nc.sync.dma_start(out=x1T_sb[:], in_=x1T[:])
            gidx_sb = cp.tile([128, B * 128], I16, tag="gidx")
            nc.sync.dma_start(out=gidx_sb[:], in_=gidx_p[:])
            scat_sb = cp.tile([128, B * 8], I16, tag="scat")
            nc.sync.dma_start(out=scat_sb[:], in_=scat_p[:])
            drel_sb = cp.tile([128, C], BF16, tag="drel")
            nc.sync.dma_start(out=drel_sb[:], in_=drel_p[:])
            iotac_sb = cp.tile([128, 1], F32, tag="iotac")
            nc.sync.dma_start(out=iotac_sb[:], in_=iotac_p[:])
            w1e_sb = cp.tile([F, RW], BF16, tag="w1e")
            nc.sync.dma_start(out=w1e_sb[:], in_=w1e_p[:])
            w2e_sb = []
            for q in range(kd):
                wt = cp.tile([128, RW], BF16, tag=f"w2e{q}")
                nc.sync.dma_start(out=wt[:], in_=w2e_p[q * 128:(q + 1) * 128, :])
                w2e_sb.append(wt)
            iota_sb = cp.tile([128, K_CH * 128], BF16, tag="iota")
            nc.sync.dma_start(out=iota_sb[:], in_=iota_p[:])
            sent_sb = cp.tile([1, RW], BF16, tag="sent")
            nc.sync.dma_start(out=sent_sb[:], in_=sent_p[:])
            ones_sb = cp.tile([128, 1], BF16, tag="ones")
            nc.vector.memset(ones_sb[:], 1.0)
            adsent_sb = cp.tile([1, H], BF16, tag="adsent")
            nc.vector.memset(adsent_sb[:], 0.0)
            lwg_sb = cp.tile([1, D], F32, tag="lwg")
            nc.sync.dma_start(out=lwg_sb[:], in_=lwg_p[:])
            lwuw_sb = cp.tile([1, 2], F32, tag="lwuw")
            nc.sync.dma_start(out=lwuw_sb[:], in_=lwuw_p[:])
            uw_sb = cp.tile([1, 2], F32, tag="uw")
            nc.sync.dma_start(out=uw_sb[:], in_=uw_p[:])
            lb_sb = cp.tile([1, 1], F32, tag="lb")
            nc.sync.dma_start(out=lb_sb[:], in_=lb_p[:])

            adb_sb = []
            for i in (1, 2):
                adb_t = cp.tile([128, B * H], BF16, tag=f"adb{i}", name=f"adb{i}")
                adb_sb.append(adb_t)
            xs_all = cp.tile([128, B * D], BF16, tag="xs_all")

            # zero x2 (scatter-add target; pad rows must stay zero)
            zt = cp.tile([128, D], BF16, tag="zpad")
            nc.vector.memset(zt[:], 0.0)
            for t in range(n_tiles):
                nc.sync.dma_start(
                    out=x2_dram[t * 128:(t + 1) * 128, :], in_=zt[:]
                )

            csum_ps = psC.tile([1, D], F32, tag="cs")
            x2T_sb = None

            for layer in range(2):
                # ---- dense ------------------------------------------------
                for t in range(n_tiles):
                    ps = psA.tile([128, RW], F32, tag="ps")
                    if layer == 0:
                        nc.tensor.matmul(
                            out=ps[:],
                            lhsT=x1T_sb[:, t * 128:(t + 1) * 128],
                            rhs=w1e_sb[:],
                            start=True, stop=True,
                        )
                    else:
                        for q in range(kd):
                            nc.tensor.matmul(
                                out=ps[:],
                                lhsT=x2T_sb[q][:, t * 128:(t + 1) * 128],
                                rhs=w2e_sb[q][:],
                                start=(q == 0), stop=(q == kd - 1),
                            )
                    stg = dstgp.tile([128, RW], BF16, tag="stg")
                    nc.vector.tensor_copy(out=stg[:], in_=ps[:])
                    r1 = SR - 1 if t == n_tiles - 1 else (t + 1) * 128
                    nr = r1 - t * 128
                    nc.sync.dma_start(
                        out=hext_own[layer][t * 128:r1, 0:RW], in_=stg[0:nr, :]
                    )
                    nc.sync.dma_start(
                        out=adp[layer][t * 128:r1, 0:H],
                        in_=stg[0:nr, RW - H:RW],
                    )
                    if t == n_tiles - 1:
                        nc.sync.dma_start(
                            out=hext_own[layer][SR - 1: SR, 0:RW], in_=sent_sb[:]
                        )
                        nc.sync.dma_start(
                            out=adp[layer][SR - 1: SR, 0:H], in_=adsent_sb[:]
                        )

                # ---- AllGather --------------------------------------------
                nc.gpsimd.collective_compute(
                    "AllGather",
                    mybir.AluOpType.bypass,
                    ins=[hext_own[layer][:]],
                    outs=[hext_full[layer][:]],
                    replica_groups=rg,
                )

                # ---- per-block slot-ad gathers (overlap the AllGather) ----
                for j in range((B + 7) // 8):
                    b0, b1 = j * 8, min((j + 1) * 8, B)
                    nb = b1 - b0
                    _dma_gather(
                        nc.gpsimd,
                        out_ap=adb_sb[layer][:, b0 * H:b1 * H].rearrange(
                            "p (c h) -> p c h", h=H
                        ),
                        in_ap=adp[layer][:, 0:H],
                        idxs_ap=scat_sb[:, b0 * 8:b1 * 8],
                        num_idxs=nb * 128,
                        elem_size=H,
                        elem_step=AD_PITCH,
                        num_reg=r_blk if nb == 8 else r_tail,
                        queue_num=j % 4,
                    )

                # ---- edge pass --------------------------------------------
                for b in range(B):
                    gblk = gp.tile([128, K_CH * RW], BF16, tag="gblk")
                    for hs in range(2):
                        _dma_gather(
                            nc.gpsimd,
                            out_ap=gblk[
                                :, hs * K_HALF * RW:(hs + 1) * K_HALF * RW
                            ].rearrange("p (c w) -> p c w", w=RW),
                            in_ap=hext_full[layer][
                                hs * HALF:(hs + 1) * HALF, 0:RW
                            ],
                            idxs_ap=gidx_sb[
                                :, b * 128 + hs * 64: b * 128 + (hs + 1) * 64
                            ],
                            num_idxs=K_HALF * 128,
                            elem_size=RW,
                            elem_step=PITCH,
                            num_reg=r_blk,
                            queue_num=(2 * b + hs) % 4,
                        )
                    # one-hot S (edges x slots) for the whole block
                    s_all = sp_.tile([128, K_CH * 128], BF16, tag="s_all")
                    nc.vector.tensor_tensor(
                        out=s_all[:].rearrange("p (g e) -> p g e", e=128),
                        in0=iota_sb[:].rearrange("p (g e) -> p g e", e=128),
                        in1=drel_sb[:, b * K_CH:(b + 1) * K_CH]
                        .unsqueeze(-1).to_broadcast([128, K_CH, 128]),
                        op=mybir.AluOpType.is_equal,
                    )
                    # one-hot S^T (slots x edges) + per-edge ad via matmul
                    dT = dTp.tile([128, K_CH * 128], U8, tag="dT")
                    nc.sync.dma_start(
                        out=dT[:], in_=drelT_p[:, b * K_CH * 128:(b + 1) * K_CH * 128]
                    )
                    sT_all = sp_.tile([128, K_CH * 128], BF16, tag="sT_all")
                    nc.vector.tensor_scalar(
                        out=sT_all[:],
                        in0=dT[:],
                        scalar1=iotac_sb[:],
                        scalar2=None,
                        op0=mybir.AluOpType.is_equal,
                    )
                    ps_ad = psAD.tile([128, K_CH * H], F32, tag="ps_ad")
                    for kk in range(K_CH):
                        nc.tensor.matmul(
                            out=ps_ad[:, kk * H:(kk + 1) * H],
                            lhsT=sT_all[:, kk * 128:(kk + 1) * 128],
                            rhs=adb_sb[layer][:, b * H:(b + 1) * H],
                            start=True, stop=True,
                        )
                    # scores: e = exp(leaky(as + ad))
                    gv = gblk[:].rearrange("p (g w) -> p g w", w=RW)
                    scc = scp.tile([128, K_CH * H], BF16, tag="scc")
                    nc.vector.tensor_tensor(
                        out=scc[:].rearrange("p (g h) -> p g h", h=H),
                        in0=gv[:, :, D:D + H],
                        in1=ps_ad[:].rearrange("p (g h) -> p g h", h=H),
                        op=mybir.AluOpType.add,
                    )
                    t2 = scp.tile([128, K_CH * H], BF16, tag="t2")
                    nc.vector.tensor_scalar_mul(
                        out=t2[:], in0=scc[:], scalar1=NEG_SLOPE
                    )
                    nc.vector.tensor_tensor(
                        out=scc[:], in0=scc[:], in1=t2[:], op=mybir.AluOpType.max
                    )
                    sce = sep.tile([128, K_CH * D], BF16, tag="sce")
                    nc.scalar.activation(
                        out=sce[:].rearrange("p (g h c) -> p g h c", h=H, c=CH),
                        in_=scc[:].rearrange("p (g h) -> p g h", h=H)
                        .unsqueeze(-1).to_broadcast([128, K_CH, H, CH]),
                        func=mybir.ActivationFunctionType.Exp,
                    )
                    nc.scalar.activation(
                        out=gv[:, :, D + H:RW],
                        in_=scc[:].rearrange("p (g h) -> p g h", h=H),
                        func=mybir.ActivationFunctionType.Exp,
                    )
                    nc.vector.tensor_tensor(
                        out=gv[:, :, 0:D],
                        in0=gv[:, :, 0:D],
                        in1=sce[:].rearrange("p (g c) -> p g c", c=D),
                        op=mybir.AluOpType.mult,
                    )
                    # aggregation + denominator in one matmul per chunk
                    ps_o = psO.tile([128, RW], F32, tag="ps_o")
                    for kk in range(K_CH):
                        nc.tensor.matmul(
                            out=ps_o[:],
                            lhsT=s_all[:, kk * 128:(kk + 1) * 128],
                            rhs=gblk[:, kk * RW:(kk + 1) * RW],
                            start=(kk == 0), stop=(kk == K_CH - 1),
                        )
                    # ---- epilogue ----------------------------------------
                    rd = epp.tile([128, H], F32, tag="rd")
                    nc.vector.tensor_scalar_add(
                        out=rd[:], in0=ps_o[:, D + H:RW], scalar1=1e-16
                    )
                    nc.vector.reciprocal(out=rd[:], in_=rd[:])
                    rde = epp.tile([128, D], F32, tag="rde")
                    nc.scalar.activation(
                        out=rde[:],
                        in_=rd[:].unsqueeze(-1).to_broadcast([128, H, CH]),
                        func=mybir.ActivationFunctionType.Copy,
                    )
                    st = epp.tile([128, D], F32, tag="st")
                    nc.vector.tensor_tensor(
                        out=st[:], in0=ps_o[:, 0:D], in1=rde[:],
                        op=mybir.AluOpType.mult,
                    )
                    tm = epp.tile([128, D], F32, tag="tm")
                    nc.vector.tensor_scalar_min(out=tm[:], in0=st[:], scalar1=0.0)
                    nc.scalar.activation(
                        out=tm[:], in_=tm[:], func=mybir.ActivationFunctionType.Exp
                    )
                    nc.vector.tensor_scalar(
                        out=st[:], in0=st[:],
                        scalar1=0.0, scalar2=-1.0,
                        op0=mybir.AluOpType.max, op1=mybir.AluOpType.add,
                    )
                    if layer == 0:
                        nc.vector.tensor_tensor(
                            out=xs_all[:, b * D:(b + 1) * D],
                            in0=st[:], in1=tm[:], op=mybir.AluOpType.add,
                        )
                    else:
                        xs = x2p.tile([128, D], BF16, tag="xs")
                        nc.vector.tensor_tensor(
                            out=xs[:], in0=st[:], in1=tm[:], op=mybir.AluOpType.add
                        )
                        nc.tensor.matmul(
                            out=csum_ps[:],
                            lhsT=ones_sb[:],
                            rhs=xs[:],
                            start=(b == 0), stop=(b == B - 1),
                        )

                # ---- layer-1 output scatter + transpose -------------------
                if layer == 0:
                    for j in range((B + 7) // 8):
                        b0, b1 = j * 8, min((j + 1) * 8, B)
                        nb = b1 - b0
                        nc.gpsimd.dma_scatter_add(
                            out_ap=x2_dram[:],
                            in_ap=xs_all[
                                :, b0 * D:b1 * D
                            ].rearrange("p (b d) -> p b d", d=D),
                            idxs_ap=scat_sb[:, b0 * 8:b1 * 8],
                            num_idxs=nb * 128,
                            num_idxs_reg=r_blk if nb == 8 else r_tail,
                            elem_size=D,
                            queue_num=j % 4,
                        )
                    x2T_sb = []
                    for q in range(kd):
                        xt = cp.tile([128, SR], BF16, tag=f"x2T{q}")
                        nc.sync.dma_start_transpose(
                            out=xt[:], in_=x2_dram[:, q * 128:(q + 1) * 128]
                        )
                        x2T_sb.append(xt)

            # ---- readout ---------------------------------------------------
            cs_sb = fp_.tile([1, D], F32, tag="cs_sb")
            nc.vector.tensor_copy(out=cs_sb[:], in_=csum_ps[:])
            nc.sync.dma_start(out=cs_in[:], in_=cs_sb[:])
            nc.gpsimd.collective_compute(
                "AllReduce",
                mybir.AluOpType.add,
                ins=[cs_in[:]],
                outs=[cs_out[:]],
                replica_groups=rg,
            )
            cs2 = fp_.tile([1, D], F32, tag="cs2")
            nc.sync.dma_start(out=cs2[:], in_=cs_out[:])
            tg = fp_.tile([1, D], F32, tag="tg")
            acc1 = fp_.tile([1, 1], F32, tag="acc1")
            nc.vector.tensor_tensor(
                out=tg[:], in0=cs2[:], in1=lwg_sb[:], op=mybir.AluOpType.mult
            )
            nc.vector.tensor_reduce(
                out=acc1[:], in_=tg[:], axis=mybir.AxisListType.X,
                op=mybir.AluOpType.add,
            )
            t2f = fp_.tile([1, 2], F32, tag="t2f")
            acc2 = fp_.tile([1, 1], F32, tag="acc2")
            nc.vector.tensor_tensor(
                out=t2f[:], in0=uw_sb[:], in1=lwuw_sb[:], op=mybir.AluOpType.mult
            )
            nc.vector.tensor_reduce(
                out=acc2[:], in_=t2f[:], axis=mybir.AxisListType.X,
                op=mybir.AluOpType.add,
            )
            nc.vector.tensor_tensor(
                out=acc1[:], in0=acc1[:], in1=acc2[:], op=mybir.AluOpType.add
            )
            nc.vector.tensor_tensor(
                out=acc1[:], in0=acc1[:], in1=lb_sb[:], op=mybir.AluOpType.add
            )
            nc.sync.dma_start(out=out_p[:], in_=acc1[:])

    # Extended Q7 instructions (dma_gather/dma_scatter_add) live in loadable
    # libraries; insert the ModifyPoolConfig reloads and encode them to ISA
    # bytes so walrus's visitInstISA can emit them.
    import bass_rust as _bass_rust

    from concourse.library_config import all_libraries, standard
    inst_type_to_lib_mask = {}
    for lib in all_libraries:
        for inst_type in lib.instructions:
            inst_type_to_lib_mask[inst_type] = inst_type_to_lib_mask.get(
                inst_type, 0
            ) | (1 << lib.index)
    _bass_rust.insert_library_loads(
        nc, inst_type_to_lib_mask, len(all_libraries), standard.index
    )
    mybir.codegen_inst_isa_subclasses(nc)

    if LEGALIZE_WAITS:
        _legalize_waits(nc)
    return nc


# ----------------------------------------------------------------------------
# Host-side input assembly
# ----------------------------------------------------------------------------
def _att_matrix(att: np.ndarray) -> np.ndarray:
    Hh, Cc = att.shape
    A = np.zeros((Hh * Cc, Hh), dtype=np.float64)
    for h in range(Hh):
        A[h * Cc:(h + 1) * Cc, h] = att[h]
    return A


def _pack_we(W, a_s, a_d):
    """[W | W@As | W@Ad] -> packed [h(256) | as(8) | ad(8)]."""
    return np.concatenate([W, W @ _att_matrix(a_s), W @ _att_matrix(a_d)], axis=1)


def _make_inputs(prep, cfg, x, u, w, W1, as1, ad1, W2, as2, ad2, lin_w, lin_b):
    SR, NSH = prep["SR"], prep["NSH"]
    F, D, H = cfg["F"], cfg["D"], cfg["H"]
    n_nodes = x.shape[0]

    W1e = _pack_we(W1, as1, ad1).astype(NP_BF16)
    W2e = _pack_we(W2, as2, ad2).astype(NP_BF16)
    iota_rep = np.tile(np.arange(128, dtype=np.float32), (128, K_CH)).astype(
        NP_BF16
    )
    sent_row = np.zeros((1, RW), dtype=np.float32)
    sent_row[0, D:D + H] = NEG_BIG
    linw_g = (lin_w[0, :D] / float(n_nodes)).astype(np.float32).reshape(1, D)
    linw_uw = lin_w[0, D:D + 2].astype(np.float32).reshape(1, 2)
    uwv = np.array([[float(u), float(w)]], dtype=np.float32)
    lbv = np.asarray(lin_b, dtype=np.float32).reshape(1, 1)

    in_maps = []
    for k in range(N_CORES):
        lo = k * NSH
        hi = min(lo + NSH, n_nodes)
        xs = np.zeros((SR, F), dtype=np.float32)
        xs[: hi - lo] = x[lo:hi]
        m = {
            "x1T": np.ascontiguousarray(xs.T).astype(NP_BF16),
            "gidx": prep["gidx"][k],
            "scat": prep["scat"][k],
            "drel": prep["drel"][k].astype(NP_BF16),
            "drelT": prep["drelT"][k],
            "iota_col": np.arange(128, dtype=np.float32).reshape(128, 1),
            "W1e": W1e,
            "W2e": W2e,
            "iota_rep": iota_rep,
            "sent_row": sent_row.astype(NP_BF16),
            "linw_g": linw_g,
            "linw_uw": linw_uw,
            "uw": uwv,
            "lin_b": lbv,
        }
        in_maps.append(m)
    return in_maps


def build_all(x, edge_index, u, w, W1, att_src1, att_dst1, bias1,
              W2, att_src2, att_dst2, bias2, lin_w, lin_b, **_kw):
    n_nodes, F = x.shape
    H, Cc = att_src1.shape
    D = H * Cc
    assert not (np.any(np.asarray(bias1)) or np.any(np.asarray(bias2))), (
        "bias path not implemented (reference uses zero biases)"
    )
    prep = _preprocess(np.asarray(edge_index), n_nodes)
    cfg = dict(
        SR=prep["SR"], B=prep["B"], NSH=prep["NSH"], HALF=prep["HALF"],
        F=F, D=D, H=H,
    )
    nc = _build_program(cfg)
    in_maps = _make_inputs(
        prep, cfg, np.asarray(x, np.float32), u, w,
        np.asarray(W1, np.float64), np.asarray(att_src1, np.float64),
        np.asarray(att_dst1, np.float64),
        np.asarray(W2, np.float64), np.asarray(att_src2, np.float64),
        np.asarray(att_dst2, np.float64),
        np.asarray(lin_w, np.float64), np.asarray(lin_b, np.float64),
    )
    return nc, in_maps


def kernel(**inputs) -> np.ndarray:
    nc, in_maps = build_all(
        inputs["x"], inputs["edge_index"], inputs["u"], inputs["w"],
        inputs["W1"], inputs["att_src1"], inputs["att_dst1"], inputs["bias1"],
        inputs["W2"], inputs["att_src2"], inputs["att_dst2"], inputs["bias2"],
        inputs["lin_w"], inputs["lin_b"],
    )
    res = run_bass_kernel_spmd(nc, in_maps, core_ids=list(range(N_CORES)))
    return res.results[0]["out"].reshape(1).astype(np.float32)
